# revision 1
# baseline (speedup 1.0000x reference)
"""PSLoRA linear layer on 8 Trainium2 NeuronCores (Bass/Tile, float32r).

out[b] = x[b] @ W.T + bias + 0.5 * (x[b] @ lora_A[idx[b]]) @ lora_B.T

Sharding: data-parallel over batch (B=8 -> one batch element per core).
W / lora params are replicated; the per-core lora_A gather happens on host
(index has only 8 entries). Per core the kernel computes a fused GEMM:
the LoRA delta and the bias are folded into the same PSUM accumulation
group as the base matmul via an augmented K=33 matmul (32 axT rows + a
ones row paired with [0.5*B^T; bias]).

Device loop per core: 2 s-halves (x half resident in SBUF, 16 MiB);
per half: axT = A^T x^T computed into PSUM then parked in SBUF; then 8
output panels of 512 columns, each accumulating 32 K-tiles across 8 PSUM
banks (one per 128-row s-block) + 1 LoRA/bias matmul, evicted via DVE to
SBUF and DMA'd out. All matmuls are float32r: fp32 storage with
full-rate (1 cycle/row) PE streaming, ~1.5e-4 relative error.
"""
import sys
sys.path.insert(0, "/opt/trn_rl_repo")
import numpy as np

B, S, DIN, DOUT, R = 8, 2048, 4096, 4096, 32
LORA_SCALING = 16 / 32
KT = DIN // 128          # 32 contraction tiles
HALF = 1024              # s rows per resident half
NH = S // HALF
SBH = HALF // 128        # s-blocks per half
OB = DOUT // 512         # output panels
N_CORES = 8

_cache = {}


def _build():
    import concourse.bacc as bacc
    import concourse.mybir as mybir
    from concourse.tile import TileContext

    F32R = mybir.dt.float32r
    F32 = mybir.dt.float32

    nc = bacc.Bacc()
    xT = nc.dram_tensor("xT", [DIN, S], F32R, kind="ExternalInput")
    WT = nc.dram_tensor("WT", [DIN, DOUT], F32R, kind="ExternalInput")
    AbR = nc.dram_tensor("AbR", [128, KT, R], F32R, kind="ExternalInput")
    # rows 0-31: 0.5*lora_B.T, row 32: bias
    BTa = nc.dram_tensor("BTa", [R + 1, DOUT], F32R, kind="ExternalInput")
    ONES = nc.dram_tensor("ONES", [1, 512], F32R, kind="ExternalInput")
    out = nc.dram_tensor("out", [S, DOUT], F32, kind="ExternalOutput")

    with TileContext(nc) as tc:
        with (
            tc.tile_pool(name="xp", bufs=KT) as xp,
            tc.tile_pool(name="wp", bufs=8) as wp,
            tc.tile_pool(name="cp", bufs=1) as cp,
            tc.tile_pool(name="axp", bufs=2) as axp,
            tc.tile_pool(name="op", bufs=8) as op_,
            tc.tile_pool(name="pp", bufs=1, space="PSUM") as pp,
        ):
            ab = cp.tile([128, KT, R], F32R, name="ab")
            nc.sync.dma_start(ab, AbR[:, :, :])
            bt = cp.tile([R + 1, DOUT], F32R, name="bt")
            nc.sync.dma_start(bt, BTa[:, :])

            for h in range(NH):
                xt = []
                for k in range(KT):
                    t = xp.tile([128, HALF], F32R, name="xq")
                    nc.sync.dma_start(
                        t, xT[k * 128:(k + 1) * 128, h * HALF:(h + 1) * HALF])
                    xt.append(t)
                # axT (transposed lora activations) per 512-col chunk
                axc = []
                for c in range(HALF // 512):
                    pa = pp.tile([R, 512], F32, name=f"ps{c}")
                    for k in range(KT):
                        nc.tensor.matmul(
                            pa, lhsT=ab[:, k, :],
                            rhs=xt[k][:, c * 512:(c + 1) * 512],
                            start=(k == 0), stop=(k == KT - 1))
                    axt = axp.tile([R + 1, 512], F32R, name="axt")
                    nc.vector.tensor_copy(axt[0:R, :], pa)
                    nc.sync.dma_start(axt[R:R + 1, :], ONES[0:1, :])
                    axc.append(axt)
                # main panels: base matmul + fused lora delta + bias
                for ob in range(OB):
                    ps = [pp.tile([128, 512], F32, name=f"ps{sb}")
                          for sb in range(SBH)]
                    for k in range(KT):
                        wt = wp.tile([128, 512], F32R, name="wt")
                        nc.sync.dma_start(
                            wt, WT[k * 128:(k + 1) * 128,
                                   ob * 512:(ob + 1) * 512])
                        for sb in range(SBH):
                            nc.tensor.matmul(
                                ps[sb],
                                lhsT=xt[k][:, sb * 128:(sb + 1) * 128],
                                rhs=wt, start=(k == 0), stop=False)
                    for sb in range(SBH):
                        nc.tensor.matmul(
                            ps[sb],
                            lhsT=axc[sb // 4][:, (sb % 4) * 128:(sb % 4 + 1) * 128],
                            rhs=bt[:, ob * 512:(ob + 1) * 512],
                            start=False, stop=True)
                    for sb in range(SBH):
                        ot = op_.tile([128, 512], F32, name="ot")
                        nc.vector.tensor_copy(ot, ps[sb])
                        nc.scalar.dma_start(
                            out[h * HALF + sb * 128:h * HALF + (sb + 1) * 128,
                                ob * 512:(ob + 1) * 512], ot)
    nc.finalize()
    return nc


def _prep_in_maps(input, weight, bias, lora_A, lora_B, labeler_index):
    x = np.asarray(input, dtype=np.float32)
    W = np.asarray(weight, dtype=np.float32)
    bias = np.asarray(bias, dtype=np.float32)
    lA = np.asarray(lora_A, dtype=np.float32)
    lB = np.asarray(lora_B, dtype=np.float32)
    idx = np.asarray(labeler_index).astype(np.int64)

    WT = np.ascontiguousarray(W.T)
    BTa = np.ascontiguousarray(
        np.concatenate([LORA_SCALING * lB.T, bias[None, :]], axis=0),
        dtype=np.float32)
    ones = np.ones((1, 512), dtype=np.float32)

    in_maps = []
    for b in range(B):
        xTb = np.ascontiguousarray(x[b].T)
        Ab = lA[idx[b]]
        AbR = np.ascontiguousarray(Ab.reshape(KT, 128, R).transpose(1, 0, 2))
        in_maps.append({"xT": xTb, "WT": WT, "AbR": AbR, "BTa": BTa,
                        "ONES": ones})
    return in_maps


def kernel(input, weight, bias, lora_A, lora_B, labeler_index):
    from concourse import bass_utils

    in_maps = _prep_in_maps(input, weight, bias, lora_A, lora_B, labeler_index)
    if "nc" not in _cache:
        _cache["nc"] = _build()
    last_err = None
    for attempt in range(3):
        try:
            res = bass_utils.run_bass_kernel_spmd(
                _cache["nc"], in_maps, core_ids=list(range(N_CORES)))
            return np.stack([res.results[b]["out"] for b in range(B)])
        except Exception as e:  # transient NRT wedge from a prior crashed run
            last_err = e
            if "UNRECOVERABLE" not in str(e) and "UNAVAILABLE" not in str(e):
                raise
    raise last_err

